# revision 50
# baseline (speedup 1.0000x reference)
"""DotAttackHead kernel for Trainium2 (8 NeuronCores, data-parallel over batch).

prob = softmax(relu(ufeat @ W.T + b) @ efeat.T / sqrt(256) + mask_bias)
W = g * v / ||v||_F

Sharding: batch 64 -> 8 cores x 8 batches (data-parallel). Params replicated.

Host prep: weight-norm W, transpose+bf16-cast of ufeat/efeat (the PE needs
the contraction dim on partitions, and bf16 halves input DMA), and the mask
folded into efeat: masked columns (n >= num_enemy) are set to -1e30, so
masked logits land at <= -1e28 and exp underflows to exactly 0 — the same 0
the reference's -1e9 bias produces.

Device per batch b (software-pipelined across batches):
  mm1:  projT[e,u] = relu(wT.T @ ufT[b] + bias)   (PE bf16; bias+relu fused
        on DVE as tensor_scalar add/max reading PSUM, bf16 out)
  mm2:  psum[u,n]  = projT.T @ efT[b]             (PE bf16, fp32 PSUM)
  soft: e = Exp(psum/16) with accum_out row-sum for free (ACT), r = 1/s
        (DVE reciprocal), prob = e * r (DVE 4x bf16), bf16 DMA out
        (host upcasts to f32).
No max-subtraction: logits are O(+-6) so exp is safe in fp32, and softmax is
shift-invariant, so this matches the reference.

Mask-width specialization: masked output columns are exactly 0, so the
program is compiled (per num_enemy multiset, NEFF-cached) with a static
per-slot column budget: batches sorted by effective width descending,
rank 8k+c -> (core c, slot k), slot width = slot max rounded up to 128.
Only columns [0, W_k) are computed/stored; the rest of each output row is
zeroed (device writes nothing there; host also zeroes defensively).
Adjacent u-tiles share one paired [128, 2, W] store (Sync DIRECT2D issue
cost is size-independent).

Ramp: mm1 groups are emitted uc-major (both e-halves of u-chunk 0 first,
so mm2 u0..u3 unblock after 2 of 4 groups) and each uft load is split
into two u-half DMAs (the first groups start after 512KB lands, not 1MB).

Measured on trn2: ~84.7 us/core HW exec, rel err ~4.8e-3 (vs 211 us first
working version). Profile: ACT exp 44 us + accum 12, PE 63 us busy,
DVE 63 us, Sync 55 us, ~7 us fixed preamble — balanced at the ridge;
run-to-run variance is +-1.5-2.5 us.
"""

from contextlib import ExitStack

import ml_dtypes
import numpy as np

import concourse.bass as bass
import concourse.mybir as mybir
import concourse.tile as tile
from concourse import bacc
from concourse.bass_utils import run_bass_kernel_spmd

N_CORES = 8
B = 64
U = 1024  # units
E = 256   # efeat dim
K = 512   # ufeat dim
N = 1024  # enemies
BPC = B // N_CORES  # batches per core

F32 = mybir.dt.float32
BF16 = mybir.dt.bfloat16
BF16_NP = ml_dtypes.bfloat16

def _build_bass(bpc: int = BPC, widths: tuple = ()) -> bass.Bass:
    if not widths:
        widths = (N,) * bpc
    assert len(widths) == bpc and all(w % 128 == 0 and 128 <= w <= N for w in widths)
    # Bacc (not raw Bass): its finalize() runs generate_event_semaphores,
    # which splits multi-wait instructions to satisfy TRN2's 1-wait limit.
    nc = bacc.Bacc(None, target_bir_lowering=False)

    ufT = nc.declare_dram_parameter("ufT", [bpc, K, U], BF16, isOutput=False)
    efT = nc.declare_dram_parameter("efT", [bpc, E, N], BF16, isOutput=False)
    wT = nc.declare_dram_parameter("wT", [K, E], BF16, isOutput=False)
    bias = nc.declare_dram_parameter("bias", [E], F32, isOutput=False)
    # bf16 output store halves the dominant DMA stream; host upcasts to f32.
    prob = nc.declare_dram_parameter("prob", [bpc, U, N], BF16, isOutput=True)

    with tile.TileContext(nc) as tc, ExitStack() as ctx:
        singles = ctx.enter_context(tc.tile_pool(name="singles", bufs=1))
        pin = ctx.enter_context(tc.tile_pool(name="pin", bufs=5))
        pproj = ctx.enter_context(tc.tile_pool(name="pproj", bufs=3))
        pet = ctx.enter_context(tc.tile_pool(name="pet", bufs=8))
        pprob = ctx.enter_context(tc.tile_pool(name="pprob", bufs=8))
        psmall = ctx.enter_context(tc.tile_pool(name="psmall", bufs=16))
        pps1 = ctx.enter_context(tc.tile_pool(name="pps1", bufs=2, space="PSUM"))
        pps2 = ctx.enter_context(tc.tile_pool(name="pps2", bufs=3, space="PSUM"))

        # ---- resident constants ----
        # wT as 4 k-tiles: wt_sb[p, kt, e] = wT[kt*128+p, e]
        wt_sb = singles.tile([128, 4, E], BF16)
        nc.sync.dma_start(out=wt_sb, in_=wT[:, :].rearrange("(kt p) e -> p kt e", p=128))
        # bias as 2 e-tiles on partitions: b_sb[p, et] = bias[et*128+p]
        b_sb = singles.tile([128, 2], F32)
        nc.sync.dma_start(out=b_sb, in_=bias[:].rearrange("(et p) -> p et", p=128))

        def emit_loads(bi):
            # two u-half loads: mm1's first (uc=0) groups start after 512KB
            # lands instead of the full 1MB. Batch 0 (the ramp) goes finer:
            # per-(k,u-half) quarter loads with eft issued between the
            # halves, so the first mm1 matmul starts after ~320KB and eft
            # lands before the first mm2 needs it.
            uft = pin.tile([128, 4, U], BF16, tag="uft")
            if bi == 0:
                for kj in range(4):
                    nc.sync.dma_start(
                        out=uft[:, kj, 0:512],
                        in_=ufT[bi, kj * 128 : (kj + 1) * 128, 0:512],
                    )
                W = widths[bi]
                eft = pin.tile([128, 2, W], BF16, tag="eft", name=f"eft{bi}")
                nc.sync.dma_start(
                    out=eft,
                    in_=efT[bi, :, :W].rearrange("(et p) n -> p et n", p=128),
                )
                for kj in range(4):
                    nc.sync.dma_start(
                        out=uft[:, kj, 512:1024],
                        in_=ufT[bi, kj * 128 : (kj + 1) * 128, 512:1024],
                    )
                return uft, eft
            for uc in range(2):
                usl = slice(uc * 512, (uc + 1) * 512)
                nc.sync.dma_start(
                    out=uft[:, :, usl],
                    in_=ufT[bi, :, usl].rearrange("(kt p) u -> p kt u", p=128),
                )
            W = widths[bi]
            eft = pin.tile([128, 2, W], BF16, tag="eft", name=f"eft{bi}")
            nc.sync.dma_start(
                out=eft, in_=efT[bi, :, :W].rearrange("(et p) n -> p et n", p=128)
            )
            return uft, eft

        def emit_mm1_group(uft, projT, gi):
            # group gi -> (ej, uc), uc-major: both e-halves of u-chunk 0 come
            # first, so mm2 tiles u0..u3 unblock after 2 groups instead of 4
            ej, uc = gi % 2, gi // 2
            esl = slice(ej * 128, (ej + 1) * 128)
            usl = slice(uc * 512, (uc + 1) * 512)
            ps1 = pps1.tile([128, 512], F32, tag="ps1")
            for kj in range(4):
                nc.tensor.matmul(
                    ps1,
                    lhsT=wt_sb[:, kj, esl],
                    rhs=uft[:, kj, usl],
                    start=(kj == 0),
                    stop=(kj == 3),
                )
            # relu(x + b) = max(x + b, 0) fused on DVE; casts to bf16
            nc.vector.tensor_scalar(
                out=projT[:, ej, usl],
                in0=ps1,
                scalar1=b_sb[:, ej : ej + 1],
                scalar2=0.0,
                op0=mybir.AluOpType.add,
                op1=mybir.AluOpType.max,
            )

        pair_state = {}

        def emit_softmax_tile(bi, projT, eft, ui):
            # only the first widths[bi] columns are live (the rest of the
            # output row stays 0 via the zero-donated output buffer)
            W = widths[bi]
            nslices = [slice(0, min(512, W))] + ([slice(512, W)] if W > 512 else [])
            uslice = slice(ui * 128, (ui + 1) * 128)
            ps2 = pps2.tile([128, W], F32, tag="ps2", name=f"ps2_{bi}_{ui}")
            # e-major: consecutive matmuls share the same lhsT (weight reuse)
            for ej in range(2):
                for nsl in nslices:
                    nc.tensor.matmul(
                        ps2[:, nsl],
                        lhsT=projT[:, ej, uslice],
                        rhs=eft[:, ej, nsl],
                        start=(ej == 0),
                        stop=(ej == 1),
                    )
            et = pet.tile([128, W], BF16, tag="et", name=f"et{bi}_{ui}")
            s = psmall.tile([128, 1], F32, tag="s")
            nc.scalar.activation(
                out=et,
                in_=ps2,
                func=mybir.ActivationFunctionType.Exp,
                scale=1.0 / 16.0,
                accum_out=s,
            )
            r = psmall.tile([128, 1], F32, tag="r")
            nc.vector.reciprocal(out=r, in_=s)
            # pair adjacent u-tiles into one [128, 2, W] store tile: halves
            # the Sync DIRECT2D issue count (the per-DMA cost is size-indep)
            if ui % 2 == 0:
                pair_state["tile"] = pprob.tile(
                    [128, 2, W], BF16, tag="prob", name=f"prob{bi}_{ui}"
                )
            prob_t = pair_state["tile"]
            nc.vector.tensor_scalar_mul(out=prob_t[:, ui % 2, :], in0=et, scalar1=r)
            if ui % 2 == 1:
                base = (ui - 1) * 128
                nc.sync.dma_start(
                    out=prob[bi, base : base + 256, :W].rearrange(
                        "(j p) n -> p j n", p=128
                    ),
                    in_=prob_t,
                )

        # Software-pipelined emission: mm1 groups for batch bi+1 are emitted
        # between softmax tiles of batch bi's second half, so the PE never
        # monopolizes a contiguous ~4us window on mm1 while ACT's 3-deep
        # PSUM backlog drains.
        tiles = {0: emit_loads(0)}
        projs = {0: pproj.tile([128, 2, U], BF16, tag="projT", name="projT0")}
        for gi in range(4):
            emit_mm1_group(tiles[0][0], projs[0], gi)
        for bi in range(bpc):
            uft, eft = tiles[bi]
            projT = projs[bi]
            if bi + 1 < bpc:
                tiles[bi + 1] = emit_loads(bi + 1)
            for ui in range(4):
                emit_softmax_tile(bi, projT, eft, ui)
            if bi + 1 < bpc:
                projs[bi + 1] = pproj.tile(
                    [128, 2, U], BF16, tag="projT", name=f"projT{bi + 1}"
                )
            # mm1 groups for bi+1 ride along u4..u7 so the PE never
            # monopolizes a contiguous ~4us window on mm1 while ACT's
            # 3-deep PSUM backlog drains
            for ui in range(4, 8):
                emit_softmax_tile(bi, projT, eft, ui)
                if bi + 1 < bpc:
                    emit_mm1_group(tiles[bi + 1][0], projs[bi + 1], ui - 4)

    # Runs Bacc.compile(): register allocation + event-semaphore splitting.
    nc.finalize()
    return nc


def _prep_inputs(ufeat, efeat, num_enemy, v, g, b):
    """Host-side prep: weight-norm, transpose + bf16 cast, mask bias."""
    ufeat = np.asarray(ufeat, dtype=np.float32)
    efeat = np.asarray(efeat, dtype=np.float32)
    num_enemy = np.asarray(num_enemy).astype(np.int64)
    v = np.asarray(v, dtype=np.float32)
    g = np.float32(np.asarray(g))
    b = np.asarray(b, dtype=np.float32)

    W = (g / np.float32(np.linalg.norm(v))) * v  # [E, K]
    wT = np.ascontiguousarray(W.T).astype(BF16_NP)  # [K, E]

    # [B, K, U] / [B, E, N] bf16 (cast first: halves the transpose traffic)
    ufT = ufeat.astype(BF16_NP).transpose(0, 2, 1)
    efT = np.ascontiguousarray(efeat.astype(BF16_NP).transpose(0, 2, 1))

    # Mask: poison masked efeat columns (n >= num_enemy) with -1e30. Since
    # proj >= 0 (relu) and a proj row is never identically 0 in practice,
    # masked logits land at <= -1e28 and exp underflows to exactly 0 — the
    # same 0 the reference's -1e9 bias produces. num_enemy==0 => all lanes
    # masked => the reference's uniform -1e9 shift cancels in softmax =>
    # leave those batches unpoisoned.
    ne = np.where(num_enemy > 0, num_enemy, N)
    col_masked = np.arange(N)[None, :] >= ne[:, None]  # [B, N]
    efT[np.broadcast_to(col_masked[:, None, :], efT.shape)] = BF16_NP(-1e30)

    return ufT, efT, wT, b


_nc_cache: dict[tuple, bass.Bass] = {}


def run(ufeat, efeat, num_enemy, v, g, b, trace=False):
    ufT, efT, wT, b = _prep_inputs(ufeat, efeat, num_enemy, v, g, b)

    # Masked columns (n >= num_enemy) of the output are exactly 0 and the
    # PJRT path donates zero-initialized output buffers, so the kernel only
    # needs to compute/store columns [0, W) per batch. Sort batches by
    # effective width (descending), assign rank 8k+c to (core c, slot k),
    # and compile the program with a static per-slot width = the slot's max
    # rounded up to 128. Identical widths across cores keeps it SPMD.
    ne = np.asarray(num_enemy).astype(np.int64)
    ne_eff = np.where(ne > 0, ne, N)
    order = np.argsort(-ne_eff, kind="stable")  # descending: widest slot
    # first (overlaps the ramp), narrowest last (short drain tail)
    slot_ne = ne_eff[order].reshape(BPC, N_CORES)
    widths = tuple(
        int(max(128, -(-int(m) // 128) * 128)) for m in slot_ne.max(axis=1)
    )

    key = (BPC, widths)
    if key not in _nc_cache:
        _nc_cache[key] = _build_bass(BPC, widths)
    nc = _nc_cache[key]

    in_maps = []
    perms = []
    for c in range(N_CORES):
        perm = order.reshape(BPC, N_CORES)[:, c]  # batch index for each slot
        perms.append(perm)
        in_maps.append({"ufT": ufT[perm], "efT": efT[perm], "wT": wT, "bias": b})

    res = run_bass_kernel_spmd(nc, in_maps, list(range(N_CORES)), trace=trace)
    out = np.empty((B, U, N), dtype=np.float32)
    for c in range(N_CORES):
        o = res.results[c]["prob"].astype(np.float32)
        # Columns [W, N) are masked => exactly 0 by construction. The device
        # never writes them (donated output buffers are zero-filled); zero
        # them here too so correctness never rests on buffer-init behavior.
        for k, w in enumerate(widths):
            o[k, :, w:] = 0.0
        out[perms[c]] = o
    return out, res


def kernel(ufeat, efeat, num_enemy, v, g, b):
    out, _ = run(ufeat, efeat, num_enemy, v, g, b, trace=False)
    return out


# revision 51
# speedup vs baseline: 1.0027x; 1.0027x over previous
"""DotAttackHead kernel for Trainium2 (8 NeuronCores, data-parallel over batch).

prob = softmax(relu(ufeat @ W.T + b) @ efeat.T / sqrt(256) + mask_bias)
W = g * v / ||v||_F

Sharding: batch 64 -> 8 cores x 8 batches (data-parallel). Params replicated.

Host prep: weight-norm W, transpose+bf16-cast of ufeat/efeat (the PE needs
the contraction dim on partitions, and bf16 halves input DMA), and the mask
folded into efeat: masked columns (n >= num_enemy) are set to -1e30, so
masked logits land at <= -1e28 and exp underflows to exactly 0 — the same 0
the reference's -1e9 bias produces.

Device per batch b (software-pipelined across batches):
  mm1:  projT[e,u] = relu(wT.T @ ufT[b] + bias)   (PE bf16; bias+relu fused
        on DVE as tensor_scalar add/max reading PSUM, bf16 out)
  mm2:  psum[u,n]  = projT.T @ efT[b]             (PE bf16, fp32 PSUM)
  soft: e = Exp(psum/16) with accum_out row-sum for free (ACT), r = 1/s
        (DVE reciprocal), prob = e * r (DVE 4x bf16), bf16 DMA out
        (host upcasts to f32).
No max-subtraction: logits are O(+-6) so exp is safe in fp32, and softmax is
shift-invariant, so this matches the reference.

Mask-width specialization: masked output columns are exactly 0, so the
program is compiled (per num_enemy multiset, NEFF-cached) with a static
per-slot column budget: batches sorted by effective width descending,
rank 8k+c -> (core c, slot k), slot width = slot max rounded up to 128.
Only columns [0, W_k) are computed/stored; the rest of each output row is
zeroed (device writes nothing there; host also zeroes defensively).
Adjacent u-tiles share one paired [128, 2, W] store (Sync DIRECT2D issue
cost is size-independent).

Ramp: mm1 groups are emitted uc-major (both e-halves of u-chunk 0 first,
so mm2 u0..u3 unblock after 2 of 4 groups) and each uft load is split
into two u-half DMAs (the first groups start after 512KB lands, not 1MB).

Measured on trn2: ~84.7 us/core HW exec, rel err ~4.8e-3 (vs 211 us first
working version). Profile: ACT exp 44 us + accum 12, PE 63 us busy,
DVE 63 us, Sync 55 us, ~7 us fixed preamble — balanced at the ridge;
run-to-run variance is +-1.5-2.5 us.
"""

from contextlib import ExitStack

import ml_dtypes
import numpy as np

import concourse.bass as bass
import concourse.mybir as mybir
import concourse.tile as tile
from concourse import bacc
from concourse.bass_utils import run_bass_kernel_spmd

N_CORES = 8
B = 64
U = 1024  # units
E = 256   # efeat dim
K = 512   # ufeat dim
N = 1024  # enemies
BPC = B // N_CORES  # batches per core

F32 = mybir.dt.float32
BF16 = mybir.dt.bfloat16
BF16_NP = ml_dtypes.bfloat16

def _build_bass(bpc: int = BPC, widths: tuple = ()) -> bass.Bass:
    if not widths:
        widths = (N,) * bpc
    assert len(widths) == bpc and all(w % 128 == 0 and 128 <= w <= N for w in widths)
    # Bacc (not raw Bass): its finalize() runs generate_event_semaphores,
    # which splits multi-wait instructions to satisfy TRN2's 1-wait limit.
    nc = bacc.Bacc(None, target_bir_lowering=False)

    ufT = nc.declare_dram_parameter("ufT", [bpc, K, U], BF16, isOutput=False)
    efT = nc.declare_dram_parameter("efT", [bpc, E, N], BF16, isOutput=False)
    wT = nc.declare_dram_parameter("wT", [K, E], BF16, isOutput=False)
    bias = nc.declare_dram_parameter("bias", [E], F32, isOutput=False)
    # bf16 output store halves the dominant DMA stream; host upcasts to f32.
    prob = nc.declare_dram_parameter("prob", [bpc, U, N], BF16, isOutput=True)

    with tile.TileContext(nc) as tc, ExitStack() as ctx:
        singles = ctx.enter_context(tc.tile_pool(name="singles", bufs=1))
        pin = ctx.enter_context(tc.tile_pool(name="pin", bufs=5))
        pproj = ctx.enter_context(tc.tile_pool(name="pproj", bufs=3))
        pet = ctx.enter_context(tc.tile_pool(name="pet", bufs=8))
        pprob = ctx.enter_context(tc.tile_pool(name="pprob", bufs=8))
        psmall = ctx.enter_context(tc.tile_pool(name="psmall", bufs=16))
        pps1 = ctx.enter_context(tc.tile_pool(name="pps1", bufs=2, space="PSUM"))
        pps2 = ctx.enter_context(tc.tile_pool(name="pps2", bufs=3, space="PSUM"))

        # ---- resident constants ----
        # wT as 4 k-tiles: wt_sb[p, kt, e] = wT[kt*128+p, e]
        wt_sb = singles.tile([128, 4, E], BF16)
        nc.sync.dma_start(out=wt_sb, in_=wT[:, :].rearrange("(kt p) e -> p kt e", p=128))
        # bias as 2 e-tiles on partitions: b_sb[p, et] = bias[et*128+p]
        b_sb = singles.tile([128, 2], F32)
        nc.sync.dma_start(out=b_sb, in_=bias[:].rearrange("(et p) -> p et", p=128))

        def emit_loads(bi):
            # two u-half loads: mm1's first (uc=0) groups start after 512KB
            # lands instead of the full 1MB
            uft = pin.tile([128, 4, U], BF16, tag="uft")
            for uc in range(2):
                usl = slice(uc * 512, (uc + 1) * 512)
                nc.sync.dma_start(
                    out=uft[:, :, usl],
                    in_=ufT[bi, :, usl].rearrange("(kt p) u -> p kt u", p=128),
                )
            W = widths[bi]
            eft = pin.tile([128, 2, W], BF16, tag="eft", name=f"eft{bi}")
            nc.sync.dma_start(
                out=eft, in_=efT[bi, :, :W].rearrange("(et p) n -> p et n", p=128)
            )
            return uft, eft

        def emit_mm1_group(uft, projT, gi):
            # group gi -> (ej, uc), uc-major: both e-halves of u-chunk 0 come
            # first, so mm2 tiles u0..u3 unblock after 2 groups instead of 4
            ej, uc = gi % 2, gi // 2
            esl = slice(ej * 128, (ej + 1) * 128)
            usl = slice(uc * 512, (uc + 1) * 512)
            ps1 = pps1.tile([128, 512], F32, tag="ps1")
            for kj in range(4):
                nc.tensor.matmul(
                    ps1,
                    lhsT=wt_sb[:, kj, esl],
                    rhs=uft[:, kj, usl],
                    start=(kj == 0),
                    stop=(kj == 3),
                )
            # relu(x + b) = max(x + b, 0) fused on DVE; casts to bf16
            nc.vector.tensor_scalar(
                out=projT[:, ej, usl],
                in0=ps1,
                scalar1=b_sb[:, ej : ej + 1],
                scalar2=0.0,
                op0=mybir.AluOpType.add,
                op1=mybir.AluOpType.max,
            )

        pair_state = {}

        def emit_softmax_tile(bi, projT, eft, ui):
            # only the first widths[bi] columns are live (the rest of the
            # output row stays 0 via the zero-donated output buffer)
            W = widths[bi]
            nslices = [slice(0, min(512, W))] + ([slice(512, W)] if W > 512 else [])
            uslice = slice(ui * 128, (ui + 1) * 128)
            ps2 = pps2.tile([128, W], F32, tag="ps2", name=f"ps2_{bi}_{ui}")
            # e-major: consecutive matmuls share the same lhsT (weight reuse)
            for ej in range(2):
                for nsl in nslices:
                    nc.tensor.matmul(
                        ps2[:, nsl],
                        lhsT=projT[:, ej, uslice],
                        rhs=eft[:, ej, nsl],
                        start=(ej == 0),
                        stop=(ej == 1),
                    )
            et = pet.tile([128, W], BF16, tag="et", name=f"et{bi}_{ui}")
            s = psmall.tile([128, 1], F32, tag="s")
            nc.scalar.activation(
                out=et,
                in_=ps2,
                func=mybir.ActivationFunctionType.Exp,
                scale=1.0 / 16.0,
                accum_out=s,
            )
            r = psmall.tile([128, 1], F32, tag="r")
            nc.vector.reciprocal(out=r, in_=s)
            # pair adjacent u-tiles into one [128, 2, W] store tile: halves
            # the Sync DIRECT2D issue count (the per-DMA cost is size-indep)
            if ui % 2 == 0:
                pair_state["tile"] = pprob.tile(
                    [128, 2, W], BF16, tag="prob", name=f"prob{bi}_{ui}"
                )
            prob_t = pair_state["tile"]
            nc.vector.tensor_scalar_mul(out=prob_t[:, ui % 2, :], in0=et, scalar1=r)
            if ui % 2 == 1:
                base = (ui - 1) * 128
                nc.sync.dma_start(
                    out=prob[bi, base : base + 256, :W].rearrange(
                        "(j p) n -> p j n", p=128
                    ),
                    in_=prob_t,
                )

        # Software-pipelined emission: mm1 groups for batch bi+1 are emitted
        # between softmax tiles of batch bi's second half, so the PE never
        # monopolizes a contiguous ~4us window on mm1 while ACT's 3-deep
        # PSUM backlog drains.
        tiles = {0: emit_loads(0)}
        projs = {0: pproj.tile([128, 2, U], BF16, tag="projT", name="projT0")}
        for gi in range(4):
            emit_mm1_group(tiles[0][0], projs[0], gi)
        for bi in range(bpc):
            uft, eft = tiles[bi]
            projT = projs[bi]
            if bi + 1 < bpc:
                tiles[bi + 1] = emit_loads(bi + 1)
            for ui in range(4):
                emit_softmax_tile(bi, projT, eft, ui)
            if bi + 1 < bpc:
                projs[bi + 1] = pproj.tile(
                    [128, 2, U], BF16, tag="projT", name=f"projT{bi + 1}"
                )
            # mm1 groups for bi+1 ride along u4..u7 so the PE never
            # monopolizes a contiguous ~4us window on mm1 while ACT's
            # 3-deep PSUM backlog drains
            for ui in range(4, 8):
                emit_softmax_tile(bi, projT, eft, ui)
                if bi + 1 < bpc:
                    emit_mm1_group(tiles[bi + 1][0], projs[bi + 1], ui - 4)

    # Runs Bacc.compile(): register allocation + event-semaphore splitting.
    nc.finalize()
    return nc


def _prep_inputs(ufeat, efeat, num_enemy, v, g, b):
    """Host-side prep: weight-norm, transpose + bf16 cast, mask bias."""
    ufeat = np.asarray(ufeat, dtype=np.float32)
    efeat = np.asarray(efeat, dtype=np.float32)
    num_enemy = np.asarray(num_enemy).astype(np.int64)
    v = np.asarray(v, dtype=np.float32)
    g = np.float32(np.asarray(g))
    b = np.asarray(b, dtype=np.float32)

    W = (g / np.float32(np.linalg.norm(v))) * v  # [E, K]
    wT = np.ascontiguousarray(W.T).astype(BF16_NP)  # [K, E]

    # [B, K, U] / [B, E, N] bf16 (cast first: halves the transpose traffic)
    ufT = ufeat.astype(BF16_NP).transpose(0, 2, 1)
    efT = np.ascontiguousarray(efeat.astype(BF16_NP).transpose(0, 2, 1))

    # Mask: poison masked efeat columns (n >= num_enemy) with -1e30. Since
    # proj >= 0 (relu) and a proj row is never identically 0 in practice,
    # masked logits land at <= -1e28 and exp underflows to exactly 0 — the
    # same 0 the reference's -1e9 bias produces. num_enemy==0 => all lanes
    # masked => the reference's uniform -1e9 shift cancels in softmax =>
    # leave those batches unpoisoned.
    ne = np.where(num_enemy > 0, num_enemy, N)
    col_masked = np.arange(N)[None, :] >= ne[:, None]  # [B, N]
    efT[np.broadcast_to(col_masked[:, None, :], efT.shape)] = BF16_NP(-1e30)

    return ufT, efT, wT, b


_nc_cache: dict[tuple, bass.Bass] = {}


def run(ufeat, efeat, num_enemy, v, g, b, trace=False):
    ufT, efT, wT, b = _prep_inputs(ufeat, efeat, num_enemy, v, g, b)

    # Masked columns (n >= num_enemy) of the output are exactly 0 and the
    # PJRT path donates zero-initialized output buffers, so the kernel only
    # needs to compute/store columns [0, W) per batch. Sort batches by
    # effective width (descending), assign rank 8k+c to (core c, slot k),
    # and compile the program with a static per-slot width = the slot's max
    # rounded up to 128. Identical widths across cores keeps it SPMD.
    ne = np.asarray(num_enemy).astype(np.int64)
    ne_eff = np.where(ne > 0, ne, N)
    order = np.argsort(-ne_eff, kind="stable")  # descending: widest slot
    # first (overlaps the ramp), narrowest last (short drain tail)
    slot_ne = ne_eff[order].reshape(BPC, N_CORES)
    widths = tuple(
        int(max(128, -(-int(m) // 128) * 128)) for m in slot_ne.max(axis=1)
    )

    key = (BPC, widths)
    if key not in _nc_cache:
        _nc_cache[key] = _build_bass(BPC, widths)
    nc = _nc_cache[key]

    in_maps = []
    perms = []
    for c in range(N_CORES):
        perm = order.reshape(BPC, N_CORES)[:, c]  # batch index for each slot
        perms.append(perm)
        in_maps.append({"ufT": ufT[perm], "efT": efT[perm], "wT": wT, "bias": b})

    res = run_bass_kernel_spmd(nc, in_maps, list(range(N_CORES)), trace=trace)
    out = np.empty((B, U, N), dtype=np.float32)
    for c in range(N_CORES):
        o = res.results[c]["prob"].astype(np.float32)
        # Columns [W, N) are masked => exactly 0 by construction. The device
        # never writes them (donated output buffers are zero-filled); zero
        # them here too so correctness never rests on buffer-init behavior.
        for k, w in enumerate(widths):
            o[k, :, w:] = 0.0
        out[perms[c]] = o
    return out, res


def kernel(ufeat, efeat, num_enemy, v, g, b):
    out, _ = run(ufeat, efeat, num_enemy, v, g, b, trace=False)
    return out


# revision 53
# speedup vs baseline: 1.0269x; 1.0241x over previous
"""DotAttackHead kernel for Trainium2 (8 NeuronCores, data-parallel over batch).

prob = softmax(relu(ufeat @ W.T + b) @ efeat.T / sqrt(256) + mask_bias)
W = g * v / ||v||_F

Sharding: batch 64 -> 8 cores x 8 batches (data-parallel). Params replicated.

Host prep: weight-norm W, transpose+bf16-cast of ufeat/efeat (the PE needs
the contraction dim on partitions, and bf16 halves input DMA), and the mask
folded into efeat: masked columns (n >= num_enemy) are set to -1e30, so
masked logits land at <= -1e28 and exp underflows to exactly 0 — the same 0
the reference's -1e9 bias produces.

Device per batch b (software-pipelined across batches):
  mm1:  projT[e,u] = relu(wT.T @ ufT[b] + bias)   (PE bf16; bias+relu fused
        on DVE as tensor_scalar add/max reading PSUM, bf16 out)
  mm2:  psum[u,n]  = projT.T @ efT[b]             (PE bf16, fp32 PSUM)
  soft: e = Exp(psum/16) with accum_out row-sum for free (ACT), r = 1/s
        (DVE reciprocal), prob = e * r (DVE 4x bf16), bf16 DMA out
        (host upcasts to f32).
No max-subtraction: logits are O(+-6) so exp is safe in fp32, and softmax is
shift-invariant, so this matches the reference.

Mask-width specialization: masked output columns are exactly 0, so the
program is compiled (per num_enemy multiset, NEFF-cached) with a static
per-slot column budget: batches sorted by effective width descending,
rank 8k+c -> (core c, slot k), slot width = slot max rounded up to 128.
Only columns [0, W_k) are computed/stored; the rest of each output row is
zeroed (device writes nothing there; host also zeroes defensively).
Adjacent u-tiles share one paired [128, 2, W] store (Sync DIRECT2D issue
cost is size-independent).

Ramp: mm1 groups are emitted uc-major (both e-halves of u-chunk 0 first,
so mm2 u0..u3 unblock after 2 of 4 groups) and each uft load is split
into two u-half DMAs (the first groups start after 512KB lands, not 1MB).

Measured on trn2: ~84.7 us/core HW exec, rel err ~4.8e-3 (vs 211 us first
working version). Profile: ACT exp 44 us + accum 12, PE 63 us busy,
DVE 63 us, Sync 55 us, ~7 us fixed preamble — balanced at the ridge;
run-to-run variance is +-1.5-2.5 us.
"""

from contextlib import ExitStack

import ml_dtypes
import numpy as np

import concourse.bass as bass
import concourse.mybir as mybir
import concourse.tile as tile
from concourse import bacc
from concourse.bass_utils import run_bass_kernel_spmd

N_CORES = 8
B = 64
U = 1024  # units
E = 256   # efeat dim
K = 512   # ufeat dim
N = 1024  # enemies
BPC = B // N_CORES  # batches per core

F32 = mybir.dt.float32
BF16 = mybir.dt.bfloat16
BF16_NP = ml_dtypes.bfloat16

def _build_bass(bpc: int = BPC, widths: tuple = ()) -> bass.Bass:
    if not widths:
        widths = (N,) * bpc
    assert len(widths) == bpc and all(w % 128 == 0 and 128 <= w <= N for w in widths)
    # Bacc (not raw Bass): its finalize() runs generate_event_semaphores,
    # which splits multi-wait instructions to satisfy TRN2's 1-wait limit.
    nc = bacc.Bacc(None, target_bir_lowering=False)

    ufT = nc.declare_dram_parameter("ufT", [bpc, K, U], BF16, isOutput=False)
    efT = nc.declare_dram_parameter("efT", [bpc, E, N], BF16, isOutput=False)
    wT = nc.declare_dram_parameter("wT", [K, E], BF16, isOutput=False)
    bias = nc.declare_dram_parameter("bias", [E], F32, isOutput=False)
    # bf16 output store halves the dominant DMA stream; host upcasts to f32.
    prob = nc.declare_dram_parameter("prob", [bpc, U, N], BF16, isOutput=True)

    with tile.TileContext(nc) as tc, ExitStack() as ctx:
        singles = ctx.enter_context(tc.tile_pool(name="singles", bufs=1))
        pin = ctx.enter_context(tc.tile_pool(name="pin", bufs=5))
        pproj = ctx.enter_context(tc.tile_pool(name="pproj", bufs=3))
        pet = ctx.enter_context(tc.tile_pool(name="pet", bufs=8))
        pprob = ctx.enter_context(tc.tile_pool(name="pprob", bufs=3))
        psmall = ctx.enter_context(tc.tile_pool(name="psmall", bufs=16))
        pps1 = ctx.enter_context(tc.tile_pool(name="pps1", bufs=2, space="PSUM"))
        pps2 = ctx.enter_context(tc.tile_pool(name="pps2", bufs=3, space="PSUM"))

        # ---- resident constants ----
        # wT as 4 k-tiles: wt_sb[p, kt, e] = wT[kt*128+p, e]
        wt_sb = singles.tile([128, 4, E], BF16)
        nc.sync.dma_start(out=wt_sb, in_=wT[:, :].rearrange("(kt p) e -> p kt e", p=128))
        # bias as 2 e-tiles on partitions: b_sb[p, et] = bias[et*128+p]
        b_sb = singles.tile([128, 2], F32)
        nc.sync.dma_start(out=b_sb, in_=bias[:].rearrange("(et p) -> p et", p=128))

        def emit_loads(bi):
            # two u-half loads: mm1's first (uc=0) groups start after 512KB
            # lands instead of the full 1MB
            uft = pin.tile([128, 4, U], BF16, tag="uft")
            for uc in range(2):
                usl = slice(uc * 512, (uc + 1) * 512)
                nc.sync.dma_start(
                    out=uft[:, :, usl],
                    in_=ufT[bi, :, usl].rearrange("(kt p) u -> p kt u", p=128),
                )
            W = widths[bi]
            eft = pin.tile([128, 2, W], BF16, tag="eft", name=f"eft{bi}")
            nc.sync.dma_start(
                out=eft, in_=efT[bi, :, :W].rearrange("(et p) n -> p et n", p=128)
            )
            return uft, eft

        def emit_mm1_group(uft, projT, gi):
            # group gi -> (ej, uc), uc-major: both e-halves of u-chunk 0 come
            # first, so mm2 tiles u0..u3 unblock after 2 groups instead of 4
            ej, uc = gi % 2, gi // 2
            esl = slice(ej * 128, (ej + 1) * 128)
            usl = slice(uc * 512, (uc + 1) * 512)
            ps1 = pps1.tile([128, 512], F32, tag="ps1")
            for kj in range(4):
                nc.tensor.matmul(
                    ps1,
                    lhsT=wt_sb[:, kj, esl],
                    rhs=uft[:, kj, usl],
                    start=(kj == 0),
                    stop=(kj == 3),
                )
            # relu(x + b) = max(x + b, 0) fused on DVE; casts to bf16
            nc.vector.tensor_scalar(
                out=projT[:, ej, usl],
                in0=ps1,
                scalar1=b_sb[:, ej : ej + 1],
                scalar2=0.0,
                op0=mybir.AluOpType.add,
                op1=mybir.AluOpType.max,
            )

        pair_state = {}

        def emit_softmax_tile(bi, projT, eft, ui):
            # only the first widths[bi] columns are live (the rest of the
            # output row stays 0 via the zero-donated output buffer)
            W = widths[bi]
            nslices = [slice(0, min(512, W))] + ([slice(512, W)] if W > 512 else [])
            uslice = slice(ui * 128, (ui + 1) * 128)
            ps2 = pps2.tile([128, W], F32, tag="ps2", name=f"ps2_{bi}_{ui}")
            # e-major: consecutive matmuls share the same lhsT (weight reuse)
            for ej in range(2):
                for nsl in nslices:
                    nc.tensor.matmul(
                        ps2[:, nsl],
                        lhsT=projT[:, ej, uslice],
                        rhs=eft[:, ej, nsl],
                        start=(ej == 0),
                        stop=(ej == 1),
                    )
            et = pet.tile([128, W], BF16, tag="et", name=f"et{bi}_{ui}")
            s = psmall.tile([128, 1], F32, tag="s")
            nc.scalar.activation(
                out=et,
                in_=ps2,
                func=mybir.ActivationFunctionType.Exp,
                scale=1.0 / 16.0,
                accum_out=s,
            )
            r = psmall.tile([128, 1], F32, tag="r")
            nc.vector.reciprocal(out=r, in_=s)
            # gang 4 adjacent u-tiles into one [128, 4, W] store tile:
            # quarters the Sync DIRECT2D issue count (per-DMA cost is
            # size-independent) and batches the output into 1-2MB transfers
            if ui % 4 == 0:
                pair_state["tile"] = pprob.tile(
                    [128, 4, W], BF16, tag="prob", name=f"prob{bi}_{ui}"
                )
            prob_t = pair_state["tile"]
            nc.vector.tensor_scalar_mul(out=prob_t[:, ui % 4, :], in0=et, scalar1=r)
            if ui % 4 == 3:
                base = (ui - 3) * 128
                nc.sync.dma_start(
                    out=prob[bi, base : base + 512, :W].rearrange(
                        "(j p) n -> p j n", p=128
                    ),
                    in_=prob_t,
                )

        # Software-pipelined emission: mm1 groups for batch bi+1 are emitted
        # between softmax tiles of batch bi's second half, so the PE never
        # monopolizes a contiguous ~4us window on mm1 while ACT's 3-deep
        # PSUM backlog drains.
        tiles = {0: emit_loads(0)}
        projs = {0: pproj.tile([128, 2, U], BF16, tag="projT", name="projT0")}
        for gi in range(4):
            emit_mm1_group(tiles[0][0], projs[0], gi)
        for bi in range(bpc):
            uft, eft = tiles[bi]
            projT = projs[bi]
            if bi + 1 < bpc:
                tiles[bi + 1] = emit_loads(bi + 1)
            for ui in range(4):
                emit_softmax_tile(bi, projT, eft, ui)
            if bi + 1 < bpc:
                projs[bi + 1] = pproj.tile(
                    [128, 2, U], BF16, tag="projT", name=f"projT{bi + 1}"
                )
            # mm1 groups for bi+1 ride along u4..u7 so the PE never
            # monopolizes a contiguous ~4us window on mm1 while ACT's
            # 3-deep PSUM backlog drains
            for ui in range(4, 8):
                emit_softmax_tile(bi, projT, eft, ui)
                if bi + 1 < bpc:
                    emit_mm1_group(tiles[bi + 1][0], projs[bi + 1], ui - 4)

    # Runs Bacc.compile(): register allocation + event-semaphore splitting.
    nc.finalize()
    return nc


def _prep_inputs(ufeat, efeat, num_enemy, v, g, b):
    """Host-side prep: weight-norm, transpose + bf16 cast, mask bias."""
    ufeat = np.asarray(ufeat, dtype=np.float32)
    efeat = np.asarray(efeat, dtype=np.float32)
    num_enemy = np.asarray(num_enemy).astype(np.int64)
    v = np.asarray(v, dtype=np.float32)
    g = np.float32(np.asarray(g))
    b = np.asarray(b, dtype=np.float32)

    W = (g / np.float32(np.linalg.norm(v))) * v  # [E, K]
    wT = np.ascontiguousarray(W.T).astype(BF16_NP)  # [K, E]

    # [B, K, U] / [B, E, N] bf16 (cast first: halves the transpose traffic)
    ufT = ufeat.astype(BF16_NP).transpose(0, 2, 1)
    efT = np.ascontiguousarray(efeat.astype(BF16_NP).transpose(0, 2, 1))

    # Mask: poison masked efeat columns (n >= num_enemy) with -1e30. Since
    # proj >= 0 (relu) and a proj row is never identically 0 in practice,
    # masked logits land at <= -1e28 and exp underflows to exactly 0 — the
    # same 0 the reference's -1e9 bias produces. num_enemy==0 => all lanes
    # masked => the reference's uniform -1e9 shift cancels in softmax =>
    # leave those batches unpoisoned.
    ne = np.where(num_enemy > 0, num_enemy, N)
    col_masked = np.arange(N)[None, :] >= ne[:, None]  # [B, N]
    efT[np.broadcast_to(col_masked[:, None, :], efT.shape)] = BF16_NP(-1e30)

    return ufT, efT, wT, b


_nc_cache: dict[tuple, bass.Bass] = {}


def run(ufeat, efeat, num_enemy, v, g, b, trace=False):
    ufT, efT, wT, b = _prep_inputs(ufeat, efeat, num_enemy, v, g, b)

    # Masked columns (n >= num_enemy) of the output are exactly 0 and the
    # PJRT path donates zero-initialized output buffers, so the kernel only
    # needs to compute/store columns [0, W) per batch. Sort batches by
    # effective width (descending), assign rank 8k+c to (core c, slot k),
    # and compile the program with a static per-slot width = the slot's max
    # rounded up to 128. Identical widths across cores keeps it SPMD.
    ne = np.asarray(num_enemy).astype(np.int64)
    ne_eff = np.where(ne > 0, ne, N)
    order = np.argsort(-ne_eff, kind="stable")  # descending: widest slot
    # first (overlaps the ramp), narrowest last (short drain tail)
    slot_ne = ne_eff[order].reshape(BPC, N_CORES)
    widths = tuple(
        int(max(128, -(-int(m) // 128) * 128)) for m in slot_ne.max(axis=1)
    )

    key = (BPC, widths)
    if key not in _nc_cache:
        _nc_cache[key] = _build_bass(BPC, widths)
    nc = _nc_cache[key]

    in_maps = []
    perms = []
    for c in range(N_CORES):
        perm = order.reshape(BPC, N_CORES)[:, c]  # batch index for each slot
        perms.append(perm)
        in_maps.append({"ufT": ufT[perm], "efT": efT[perm], "wT": wT, "bias": b})

    res = run_bass_kernel_spmd(nc, in_maps, list(range(N_CORES)), trace=trace)
    out = np.empty((B, U, N), dtype=np.float32)
    for c in range(N_CORES):
        o = res.results[c]["prob"].astype(np.float32)
        # Columns [W, N) are masked => exactly 0 by construction. The device
        # never writes them (donated output buffers are zero-filled); zero
        # them here too so correctness never rests on buffer-init behavior.
        for k, w in enumerate(widths):
            o[k, :, w:] = 0.0
        out[perms[c]] = o
    return out, res


def kernel(ufeat, efeat, num_enemy, v, g, b):
    out, _ = run(ufeat, efeat, num_enemy, v, g, b, trace=False)
    return out
